# revision 4
# baseline (speedup 1.0000x reference)
"""Causal self-attention on 8 trn2 NeuronCores.

Full inputs in, full output out. Sharding: data-parallel over batch (B=4),
tensor-parallel over head groups (16 heads -> 2 groups of 8). core = 2*b + g.

Per-core math (T=2048, C=1024, 8 heads, D=64, group channels G=512):
  qT/kT: [64*(h%2)+d, h//2, t] layout so scores need no transposes
  scoresT[j,i] = sum_d kT[d,j] qT[d,i]   (q pre-scaled by 1/sqrt(D) on host)
  softmax without max-subtraction (scores ~ N(0,1) by construction; exp is
  exactly shift-invariant so this matches the reference softmax)
  expT row sums come free from an all-ones 65th column appended to V
  causal mask applied post-exp by multiplying the diagonal 128x128 block
  with a precomputed lower-triangle 0/1 tile (DVE, keeps gpsimd off the
  critical exp->out chain)
  out_T[d,i] = sum_j v[j,d] expT[j,i]; normalize by 1/sums; y = oT.T @ Wp

Fully fused single-region pipeline: per 512-token chunk tch we emit
QKV projections for chunk tch, then attention rows ic=tch for all head
pairs (keys 0..tch only -- causal), then the output projection for those
tokens. The Tile list scheduler then overlaps phase boundaries (exp on
Scalar, copies on DVE, r-broadcasts on GpSimd, projections filling PE
gaps). Normalization is per (pair, ic) so the kernel tail is short.

PSUM budget (8 banks): proj pool 2x[128,512] (qkv + y outputs), scores
2x[128,2,512], out-accum 2x[65,512].

Host gather: y[b] = part[2b] + part[2b+1] + b_attn_v @ W_proj + b_proj
(q/k biases are added on-device; the v bias commutes through softmax).
"""

import numpy as np
from contextlib import ExitStack

import concourse.bass as bass
import concourse.tile as tile
from concourse import bacc, mybir
from concourse.bass_utils import run_bass_kernel_spmd

P = 128
B, T, C, H = 4, 2048, 1024, 16
D = 64
HG = 8          # heads per core
G = HG * D      # 512 head channels per core
CT = C // P     # 8 contraction tiles
TCH = T // 512  # 4 chunks of 512 tokens
NT = G // P     # 4 tiles of head channels

f32 = mybir.dt.float32
bf16 = mybir.dt.bfloat16
MM_DT = bf16
EXP_DT = bf16


def build_attention(nc: bass.Bass):
    xT = nc.dram_tensor("xT", [C, T], MM_DT, kind="ExternalInput")
    wq = nc.dram_tensor("wq", [C, G], MM_DT, kind="ExternalInput")
    wk = nc.dram_tensor("wk", [C, G], MM_DT, kind="ExternalInput")
    wv = nc.dram_tensor("wv", [C, G], MM_DT, kind="ExternalInput")
    wp = nc.dram_tensor("wp", [G, C], MM_DT, kind="ExternalInput")
    bq = nc.dram_tensor("bq", [P, NT], f32, kind="ExternalInput")
    bk = nc.dram_tensor("bk", [P, NT], f32, kind="ExternalInput")
    y = nc.dram_tensor("y", [T, C], f32, kind="ExternalOutput")

    with tile.TileContext(nc) as tc, ExitStack() as ctx:
        persist = ctx.enter_context(tc.tile_pool(name="persist", bufs=1))
        qT = persist.tile([P, NT, T], MM_DT)
        kT = persist.tile([P, NT, T], MM_DT)
        v_aug = persist.tile([P, T // P, HG, D + 1], MM_DT)
        oT = persist.tile([P, NT, T], MM_DT)
        wq_sb = persist.tile([P, CT, G], MM_DT)
        wk_sb = persist.tile([P, CT, G], MM_DT)
        wv_sb = persist.tile([P, CT, G], MM_DT)
        wp_sb = persist.tile([P, NT, C], MM_DT)
        bq_sb = persist.tile([P, NT], f32)
        bk_sb = persist.tile([P, NT], f32)
        tri = persist.tile([P, 1, P], EXP_DT)

        # weight DMAs (wq first: first matmuls need it; wp needed last)
        nc.sync.dma_start(out=wq_sb, in_=wq.ap().rearrange("(ct p) g -> p ct g", p=P))
        nc.sync.dma_start(out=wk_sb, in_=wk.ap().rearrange("(ct p) g -> p ct g", p=P))
        nc.sync.dma_start(out=wv_sb, in_=wv.ap().rearrange("(ct p) g -> p ct g", p=P))
        nc.sync.dma_start(out=bq_sb, in_=bq.ap())
        nc.sync.dma_start(out=bk_sb, in_=bk.ap())
        nc.sync.dma_start(out=wp_sb, in_=wp.ap().rearrange("(nt p) c -> p nt c", p=P))

        # constants: ones column of v_aug; lower-triangle mask (tri[p,x]=x>=p)
        ones_col = persist.tile([P, 1], f32)
        nc.vector.memset(ones_col, 1.0)
        nc.vector.tensor_copy(
            out=v_aug[:, :, :, D:D + 1],
            in_=ones_col.to_broadcast([P, T // P, HG, 1]),
        )
        nc.vector.memset(tri, 1.0)
        nc.gpsimd.affine_select(
            out=tri[:, 0, :], in_=tri[:, 0, :], compare_op=mybir.AluOpType.is_ge,
            fill=0.0, base=0, channel_multiplier=-1, pattern=[[1, P]],
        )

        xpool = ctx.enter_context(tc.tile_pool(name="xpool", bufs=2))
        epool = ctx.enter_context(tc.tile_pool(name="epool", bufs=6))
        upool = ctx.enter_context(tc.tile_pool(name="upool", bufs=6))
        rpool = ctx.enter_context(tc.tile_pool(name="rpool", bufs=4))
        bpool = ctx.enter_context(tc.tile_pool(name="bpool", bufs=4))
        ypool = ctx.enter_context(tc.tile_pool(name="ypool", bufs=3))
        ps_proj = ctx.enter_context(tc.tile_pool(name="ps_proj", bufs=2, space="PSUM"))
        ps_s = ctx.enter_context(tc.tile_pool(name="ps_s", bufs=2, space="PSUM"))
        ps_o = ctx.enter_context(tc.tile_pool(name="ps_o", bufs=2, space="PSUM"))

        xT_r = xT.ap().rearrange("(ct p) t -> p ct t", p=P)

        for tch in range(TCH):
            # ---------------- QKV projections for chunk tch ----------------
            xc = xpool.tile([P, CT, 512], MM_DT, tag="xc", name=f"xc_{tch}")
            nc.sync.dma_start(out=xc, in_=xT_r[:, :, 512 * tch:512 * (tch + 1)])

            for w_sb, b_sb, dstT in ((wq_sb, bq_sb, qT), (wk_sb, bk_sb, kT)):
                for jt in range(NT):
                    ps = ps_proj.tile([P, 512], f32, tag="proj",
                                      name=f"pqk_{tch}_{jt}")
                    for ct in range(CT):
                        nc.tensor.matmul(
                            ps, w_sb[:, ct, P * jt:P * (jt + 1)], xc[:, ct, :],
                            start=(ct == 0), stop=(ct == CT - 1),
                        )
                    nc.vector.tensor_scalar_add(
                        out=dstT[:, jt, 512 * tch:512 * (tch + 1)],
                        in0=ps, scalar1=b_sb[:, jt:jt + 1],
                    )
            for tt4 in range(4):
                tt = 4 * tch + tt4
                ps = ps_proj.tile([P, 512], f32, tag="proj", name=f"pv_{tt}")
                for ct in range(CT):
                    nc.tensor.matmul(
                        ps, xc[:, ct, P * tt4:P * (tt4 + 1)], wv_sb[:, ct, :],
                        start=(ct == 0), stop=(ct == CT - 1),
                    )
                nc.vector.tensor_copy(
                    out=v_aug[:, tt, :, 0:D],
                    in_=ps.rearrange("p (h d) -> p h d", h=HG),
                )

            # ---------------- attention rows ic = tch ----------------
            ic = tch
            n_jb = 4 * ic + 4
            for g2 in range(HG // 2):  # head pairs, nt = g2
                o_ps = {}
                for hh in range(2):
                    o_ps[hh] = ps_o.tile([D + 1, 512], f32, tag="o",
                                         name=f"ops_{2 * g2 + hh}_{ic}")
                for jb in range(n_jb):
                    off = max(0, P * jb - 512 * ic)
                    s = ps_s.tile([P, 2, 512], f32, tag="s",
                                  name=f"sps_{g2}_{ic}_{jb}")
                    for hh in range(2):
                        band = 64 * hh
                        nc.tensor.matmul(
                            s[:, hh, off:],
                            kT[band:band + D, g2, P * jb:P * (jb + 1)],
                            qT[band:band + D, g2, 512 * ic + off:512 * (ic + 1)],
                            start=True, stop=True,
                        )
                    e = epool.tile([P, 2, 512], EXP_DT, tag="e",
                                   name=f"e_{g2}_{ic}_{jb}")
                    nc.scalar.activation(
                        out=e[:, :, off:], in_=s[:, :, off:],
                        func=mybir.ActivationFunctionType.Exp,
                    )
                    if P * jb >= 512 * ic:  # diagonal triangle mask
                        nc.vector.tensor_mul(
                            out=e[:, :, off:off + P],
                            in0=e[:, :, off:off + P],
                            in1=tri.to_broadcast([P, 2, P]),
                        )
                    for hh in range(2):
                        h = 2 * g2 + hh
                        nc.tensor.matmul(
                            o_ps[hh][:, off:], v_aug[:, jb, h, :],
                            e[:, hh, off:],
                            start=(jb == 0), stop=(jb == n_jb - 1),
                        )
                # normalize: oT[d, h, ic-cols] = o/sums (per pair+ic: short tail)
                for hh in range(2):
                    h = 2 * g2 + hh
                    o_u = upool.tile([D + 1, 512], f32, tag="ou",
                                     name=f"ou_{h}_{ic}")
                    nc.vector.tensor_copy(o_u, o_ps[hh])
                    rr = rpool.tile([1, 512], f32, tag="rr", name=f"rr_{h}_{ic}")
                    nc.vector.reciprocal(rr, o_u[D:D + 1, :])
                    rb = bpool.tile([D, 512], f32, tag="rb", name=f"rb_{h}_{ic}")
                    nc.gpsimd.partition_broadcast(rb, rr[0:1, :])
                    nc.vector.tensor_mul(
                        out=oT[64 * (h % 2):64 * (h % 2) + D, h // 2,
                               512 * ic:512 * (ic + 1)],
                        in0=o_u[0:D, :],
                        in1=rb,
                    )

            # ---------------- output projection for chunk ic ----------------
            for tt4 in range(4):
                tt = 4 * ic + tt4
                y_sb = ypool.tile([P, C], f32, tag="ysb", name=f"ysb_{tt}")
                for mc in range(C // 512):
                    y_ps = ps_proj.tile([P, 512], f32, tag="proj",
                                        name=f"y_{tt}_{mc}")
                    for nt in range(NT):
                        nc.tensor.matmul(
                            y_ps,
                            oT[:, nt, P * tt:P * (tt + 1)],
                            wp_sb[:, nt, 512 * mc:512 * (mc + 1)],
                            start=(nt == 0), stop=(nt == NT - 1),
                        )
                    nc.vector.tensor_copy(out=y_sb[:, 512 * mc:512 * (mc + 1)],
                                          in_=y_ps)
                nc.sync.dma_start(out=y.ap()[P * tt:P * (tt + 1), :], in_=y_sb)


_NC_CACHE = {}


def _get_nc():
    if "nc" not in _NC_CACHE:
        nc = bacc.Bacc("TRN2", debug=False, num_devices=8)
        build_attention(nc)
        nc.compile()
        _NC_CACHE["nc"] = nc
    return _NC_CACHE["nc"]


def kernel(x, W_attn, b_attn, W_proj, b_proj):
    x = np.asarray(x, dtype=np.float32)
    W_attn = np.asarray(W_attn, dtype=np.float32)
    b_attn = np.asarray(b_attn, dtype=np.float32)
    W_proj = np.asarray(W_proj, dtype=np.float32)
    b_proj = np.asarray(b_proj, dtype=np.float32)

    import ml_dtypes
    mm_np = ml_dtypes.bfloat16

    scale = 1.0 / np.sqrt(np.float32(D))
    in_maps = []
    for core in range(8):
        b, g = divmod(core, 2)
        cols = slice(G * g, G * (g + 1))
        bqs = (b_attn[0:C][cols] * scale).reshape(NT, 2, D).transpose(1, 2, 0).reshape(P, NT)
        bks = b_attn[C:2 * C][cols].reshape(NT, 2, D).transpose(1, 2, 0).reshape(P, NT)
        in_maps.append({
            "xT": np.ascontiguousarray(x[b].T).astype(mm_np),
            "wq": np.ascontiguousarray(W_attn[:, 0:C][:, cols] * scale).astype(mm_np),
            "wk": np.ascontiguousarray(W_attn[:, C:2 * C][:, cols]).astype(mm_np),
            "wv": np.ascontiguousarray(W_attn[:, 2 * C:3 * C][:, cols]).astype(mm_np),
            "wp": np.ascontiguousarray(W_proj[G * g:G * (g + 1), :]).astype(mm_np),
            "bq": np.ascontiguousarray(bqs),
            "bk": np.ascontiguousarray(bks),
        })

    res = run_bass_kernel_spmd(_get_nc(), in_maps, core_ids=list(range(8)))

    correction = b_attn[2 * C:3 * C] @ W_proj + b_proj  # [C]
    out = np.empty((B, T, C), dtype=np.float32)
    for b in range(B):
        out[b] = res.results[2 * b]["y"] + res.results[2 * b + 1]["y"] + correction
    return out


# revision 24
# speedup vs baseline: 1.3367x; 1.3367x over previous
"""Causal self-attention on 8 trn2 NeuronCores.

Full inputs in, full output out. Sharding: data-parallel over batch (B=4),
tensor-parallel over head groups (16 heads -> 2 groups of 8). core = 2*b + g.

Per-core math (T=2048, C=1024, 8 heads, D=64, group channels G=512):
  qT/kT: [64*(h%2)+d, h//2, t] layout so scores need no transposes
  scoresT[j,i] = sum_d kT[d,j] qT[d,i]   (q pre-scaled by 1/sqrt(D) on host)
  softmax without max-subtraction (scores ~ N(0,1) by construction; exp is
  exactly shift-invariant so this matches the reference softmax)
  expT row sums come free from an all-ones 65th column appended to V
  causal mask applied post-exp by multiplying the diagonal 128x128 block
  with a precomputed lower-triangle 0/1 tile (DVE, keeps gpsimd off the
  critical exp->out chain)
  out_T[d,i] = sum_j v[j,d] expT[j,i]; normalize by 1/sums; y = oT.T @ Wp

Fully fused single-region pipeline: per 512-token chunk tch we emit
QKV projections for chunk tch, then attention rows ic=tch for all head
pairs (keys 0..tch only -- causal), then the output projection for those
tokens. The Tile list scheduler then overlaps phase boundaries (exp on
Scalar, copies on DVE, r-broadcasts on GpSimd, projections filling PE
gaps). Normalization is per (pair, ic) so the kernel tail is short.

PSUM budget (8 banks): proj pool 2x[128,512] (qkv + y outputs), scores
2x[128,2,512], out-accum 2x[65,512].

Host gather: y[b] = part[2b] + part[2b+1] + b_attn_v @ W_proj + b_proj
(q/k biases are added on-device; the v bias commutes through softmax).
"""

import numpy as np
from contextlib import ExitStack

import concourse.bass as bass
import concourse.tile as tile
from concourse import bacc, mybir
from concourse.bass_utils import run_bass_kernel_spmd

P = 128
B, T, C, H = 4, 2048, 1024, 16
D = 64
HG = 8          # heads per core
G = HG * D      # 512 head channels per core
CT = C // P     # 8 contraction tiles
TCH = T // 512  # 4 chunks of 512 tokens
NT = G // P     # 4 tiles of head channels

f32 = mybir.dt.float32
bf16 = mybir.dt.bfloat16
MM_DT = bf16
EXP_DT = bf16


def build_attention(nc: bass.Bass):
    xT = nc.dram_tensor("xT", [C, T], MM_DT, kind="ExternalInput")
    wq = nc.dram_tensor("wq", [C, G], MM_DT, kind="ExternalInput")
    wk = nc.dram_tensor("wk", [C, G], MM_DT, kind="ExternalInput")
    wv = nc.dram_tensor("wv", [C, G], MM_DT, kind="ExternalInput")
    wp = nc.dram_tensor("wp", [G, C], MM_DT, kind="ExternalInput")
    bq = nc.dram_tensor("bq", [P, NT], f32, kind="ExternalInput")
    bk = nc.dram_tensor("bk", [P, NT], f32, kind="ExternalInput")
    y = nc.dram_tensor("y", [T, C], f32, kind="ExternalOutput")

    with tile.TileContext(nc) as tc, ExitStack() as ctx:
        persist = ctx.enter_context(tc.tile_pool(name="persist", bufs=1))
        qT = persist.tile([P, NT, T], MM_DT)
        kT = persist.tile([P, NT, T], MM_DT)
        v_aug = persist.tile([P, T // P, HG, D + 1], MM_DT)
        oT = persist.tile([P, NT, T], MM_DT)
        wq_sb = persist.tile([P, CT, G], MM_DT)
        wk_sb = persist.tile([P, CT, G], MM_DT)
        wv_sb = persist.tile([P, CT, G], MM_DT)
        wp_sb = persist.tile([P, NT, C], MM_DT)
        bq_sb = persist.tile([P, NT], f32)
        bk_sb = persist.tile([P, NT], f32)
        tri = persist.tile([P, 1, P], EXP_DT)

        # weight DMAs, split per-ct so they spread across DMA engines
        # (one dma_start lands on ONE engine at ~22 GB/s; 1MB would be 44us).
        # wq first: first matmuls need it; wp needed last.
        for ct in range(CT):
            nc.sync.dma_start(out=wq_sb[:, ct, :], in_=wq.ap()[P * ct:P * (ct + 1), :])
        for ct in range(CT):
            nc.sync.dma_start(out=wk_sb[:, ct, :], in_=wk.ap()[P * ct:P * (ct + 1), :])
        for ct in range(CT):
            nc.sync.dma_start(out=wv_sb[:, ct, :], in_=wv.ap()[P * ct:P * (ct + 1), :])
        nc.sync.dma_start(out=bq_sb, in_=bq.ap())
        nc.sync.dma_start(out=bk_sb, in_=bk.ap())
        for nt in range(NT):
            nc.sync.dma_start(out=wp_sb[:, nt, :], in_=wp.ap()[P * nt:P * (nt + 1), :])

        # constants: ones column of v_aug; lower-triangle mask (tri[p,x]=x>=p)
        ones_col = persist.tile([P, 1], f32)
        nc.vector.memset(ones_col, 1.0)
        nc.vector.tensor_copy(
            out=v_aug[:, :, :, D:D + 1],
            in_=ones_col.to_broadcast([P, T // P, HG, 1]),
        )
        nc.vector.memset(tri, 1.0)
        nc.gpsimd.affine_select(
            out=tri[:, 0, :], in_=tri[:, 0, :], compare_op=mybir.AluOpType.is_ge,
            fill=0.0, base=0, channel_multiplier=-1, pattern=[[1, P]],
        )

        xpool = ctx.enter_context(tc.tile_pool(name="xpool", bufs=2))
        epool = ctx.enter_context(tc.tile_pool(name="epool", bufs=6))
        upool = ctx.enter_context(tc.tile_pool(name="upool", bufs=10))
        rpool = ctx.enter_context(tc.tile_pool(name="rpool", bufs=2))
        bpool = ctx.enter_context(tc.tile_pool(name="bpool", bufs=4))
        ypool = ctx.enter_context(tc.tile_pool(name="ypool", bufs=3))
        ps_proj = ctx.enter_context(tc.tile_pool(name="ps_proj", bufs=2, space="PSUM"))
        ps_s = ctx.enter_context(tc.tile_pool(name="ps_s", bufs=2, space="PSUM"))
        ps_o = ctx.enter_context(tc.tile_pool(name="ps_o", bufs=2, space="PSUM"))

        xT_r = xT.ap().rearrange("(ct p) t -> p ct t", p=P)

        def emit_phase3(ic):
            # output projection for chunk ic (emitted one iteration late so
            # its psum-slot claims don't block the next chunk's projections
            # while waiting on oT)
            for tt4 in range(4):
                tt = 4 * ic + tt4
                y_sb = ypool.tile([P, C], f32, tag="ysb", name=f"ysb_{tt}")
                for mc in range(C // 512):
                    y_ps = ps_proj.tile([P, 512], f32, tag="proj",
                                        name=f"y_{tt}_{mc}")
                    for nt in range(NT):
                        nc.tensor.matmul(
                            y_ps,
                            oT[:, nt, P * tt:P * (tt + 1)],
                            wp_sb[:, nt, 512 * mc:512 * (mc + 1)],
                            start=(nt == 0), stop=(nt == NT - 1),
                        )
                    nc.vector.tensor_copy(out=y_sb[:, 512 * mc:512 * (mc + 1)],
                                          in_=y_ps)
                nc.sync.dma_start(out=y.ap()[P * tt:P * (tt + 1), :], in_=y_sb)

        for tch in range(TCH):
            # ---------------- QKV projections for chunk tch ----------------
            xc = xpool.tile([P, CT, 512], MM_DT, tag="xc", name=f"xc_{tch}")
            for ct in range(CT):
                nc.sync.dma_start(
                    out=xc[:, ct, :],
                    in_=xT.ap()[P * ct:P * (ct + 1), 512 * tch:512 * (tch + 1)])

            for w_sb, b_sb, dstT in ((wq_sb, bq_sb, qT), (wk_sb, bk_sb, kT)):
                for jt in range(NT):
                    ps = ps_proj.tile([P, 512], f32, tag="proj",
                                      name=f"pqk_{tch}_{jt}")
                    for ct in range(CT):
                        nc.tensor.matmul(
                            ps, w_sb[:, ct, P * jt:P * (jt + 1)], xc[:, ct, :],
                            start=(ct == 0), stop=(ct == CT - 1),
                        )
                    nc.vector.tensor_scalar_add(
                        out=dstT[:, jt, 512 * tch:512 * (tch + 1)],
                        in0=ps, scalar1=b_sb[:, jt:jt + 1],
                    )
            for tt4 in range(4):
                tt = 4 * tch + tt4
                ps = ps_proj.tile([P, 512], f32, tag="proj", name=f"pv_{tt}")
                for ct in range(CT):
                    nc.tensor.matmul(
                        ps, xc[:, ct, P * tt4:P * (tt4 + 1)], wv_sb[:, ct, :],
                        start=(ct == 0), stop=(ct == CT - 1),
                    )
                nc.vector.tensor_copy(
                    out=v_aug[:, tt, :, 0:D],
                    in_=ps.rearrange("p (h d) -> p h d", h=HG),
                )

            if tch > 0:
                emit_phase3(tch - 1)

            # ---------------- attention rows ic = tch ----------------
            # Sum rows are staged at 32-aligned partitions {0,32,64,96} so
            # gpsimd partition_broadcast can read them (Q7 quadrant rule),
            # while one DVE reciprocal instruction covers 4 rows at once
            # (recip is an 8-pass iterative op costing free-size regardless
            # of partition count -- batching by partitions is free).
            ic = tch
            n_jb = 4 * ic + 4
            last_ic = (ic == TCH - 1)
            if not last_ic:
                S_ic = rpool.tile([8, 512], f32, tag="S", name=f"S_{ic}")
                R_ic = rpool.tile([8, 512], f32, tag="R", name=f"R_{ic}")
            o_us = {}
            for g2 in range(HG // 2):  # head pairs, nt = g2
                o_ps = {}
                for hh in range(2):
                    o_ps[hh] = ps_o.tile([D + 1, 512], f32, tag="o",
                                         name=f"ops_{2 * g2 + hh}_{ic}")
                for jb in range(n_jb):
                    off = max(0, P * jb - 512 * ic)
                    s = ps_s.tile([P, 2, 512], f32, tag="s",
                                  name=f"sps_{g2}_{ic}_{jb}")
                    for hh in range(2):
                        band = 64 * hh
                        nc.tensor.matmul(
                            s[:, hh, off:],
                            kT[band:band + D, g2, P * jb:P * (jb + 1)],
                            qT[band:band + D, g2, 512 * ic + off:512 * (ic + 1)],
                            start=True, stop=True,
                        )
                    e = epool.tile([P, 2, 512], EXP_DT, tag="e",
                                   name=f"e_{g2}_{ic}_{jb}")
                    nc.scalar.activation(
                        out=e[:, :, off:], in_=s[:, :, off:],
                        func=mybir.ActivationFunctionType.Exp,
                    )
                    if P * jb >= 512 * ic:  # diagonal triangle mask
                        nc.vector.tensor_mul(
                            out=e[:, :, off:off + P],
                            in0=e[:, :, off:off + P],
                            in1=tri.to_broadcast([P, 2, P]),
                        )
                    for hh in range(2):
                        h = 2 * g2 + hh
                        nc.tensor.matmul(
                            o_ps[hh][:, off:], v_aug[:, jb, h, :],
                            e[:, hh, off:],
                            start=(jb == 0), stop=(jb == n_jb - 1),
                        )
                # drain psum; gather sum rows (last chunk: per-pair recip so
                # the kernel tail chain is short; earlier: batched per ic)
                for hh in range(2):
                    h = 2 * g2 + hh
                    o_u = upool.tile([D + 1, 512], f32, tag="ou",
                                     name=f"ou_{h}_{ic}")
                    nc.vector.tensor_copy(o_u, o_ps[hh])
                    o_us[h] = o_u
                if last_ic:
                    # per-pair normalize: short kernel tail (baseline-proven
                    # mechanics: DMA row gather, dense recip, base-0 bcast)
                    S_g = rpool.tile([2, 512], f32, tag="Sg", name=f"Sg_{g2}")
                    R_g = rpool.tile([2, 512], f32, tag="Rg", name=f"Rg_{g2}")
                    for hh in range(2):
                        nc.sync.dma_start(out=S_g[hh:hh + 1, :],
                                          in_=o_us[2 * g2 + hh][D:D + 1, :])
                    nc.vector.reciprocal(R_g, S_g)
                    for hh in range(2):
                        h = 2 * g2 + hh
                        rrow = bpool.tile([1, 512], f32, tag="rrow",
                                          name=f"rr_{h}_{ic}")
                        nc.sync.dma_start(out=rrow, in_=R_g[hh:hh + 1, :])
                        rb = bpool.tile([D, 512], f32, tag="rb",
                                        name=f"rb_{h}_{ic}")
                        nc.gpsimd.partition_broadcast(rb, rrow[0:1, :])
                        nc.vector.tensor_mul(
                            out=oT[64 * (h % 2):64 * (h % 2) + D, h // 2,
                                   512 * ic:512 * (ic + 1)],
                            in0=o_us[h][0:D, :],
                            in1=rb,
                        )
                else:
                    for hh in range(2):
                        h = 2 * g2 + hh
                        nc.sync.dma_start(out=S_ic[h:h + 1, :],
                                          in_=o_us[h][D:D + 1, :])
            if not last_ic:
                nc.vector.reciprocal(R_ic, S_ic)
                for h in range(HG):
                    rrow = bpool.tile([1, 512], f32, tag="rrow",
                                      name=f"rr_{h}_{ic}")
                    nc.sync.dma_start(out=rrow, in_=R_ic[h:h + 1, :])
                    rb = bpool.tile([D, 512], f32, tag="rb", name=f"rb_{h}_{ic}")
                    nc.gpsimd.partition_broadcast(rb, rrow[0:1, :])
                    nc.vector.tensor_mul(
                        out=oT[64 * (h % 2):64 * (h % 2) + D, h // 2,
                               512 * ic:512 * (ic + 1)],
                        in0=o_us[h][0:D, :],
                        in1=rb,
                    )

        emit_phase3(TCH - 1)


_NC_CACHE = {}


def _get_nc():
    if "nc" not in _NC_CACHE:
        nc = bacc.Bacc("TRN2", debug=False, num_devices=8)
        build_attention(nc)
        nc.compile()
        _NC_CACHE["nc"] = nc
    return _NC_CACHE["nc"]


def kernel(x, W_attn, b_attn, W_proj, b_proj):
    x = np.asarray(x, dtype=np.float32)
    W_attn = np.asarray(W_attn, dtype=np.float32)
    b_attn = np.asarray(b_attn, dtype=np.float32)
    W_proj = np.asarray(W_proj, dtype=np.float32)
    b_proj = np.asarray(b_proj, dtype=np.float32)

    import ml_dtypes
    mm_np = ml_dtypes.bfloat16

    scale = 1.0 / np.sqrt(np.float32(D))
    in_maps = []
    for core in range(8):
        b, g = divmod(core, 2)
        cols = slice(G * g, G * (g + 1))
        bqs = (b_attn[0:C][cols] * scale).reshape(NT, 2, D).transpose(1, 2, 0).reshape(P, NT)
        bks = b_attn[C:2 * C][cols].reshape(NT, 2, D).transpose(1, 2, 0).reshape(P, NT)
        in_maps.append({
            "xT": np.ascontiguousarray(x[b].T).astype(mm_np),
            "wq": np.ascontiguousarray(W_attn[:, 0:C][:, cols] * scale).astype(mm_np),
            "wk": np.ascontiguousarray(W_attn[:, C:2 * C][:, cols]).astype(mm_np),
            "wv": np.ascontiguousarray(W_attn[:, 2 * C:3 * C][:, cols]).astype(mm_np),
            "wp": np.ascontiguousarray(W_proj[G * g:G * (g + 1), :]).astype(mm_np),
            "bq": np.ascontiguousarray(bqs),
            "bk": np.ascontiguousarray(bks),
        })

    res = run_bass_kernel_spmd(_get_nc(), in_maps, core_ids=list(range(8)))

    correction = b_attn[2 * C:3 * C] @ W_proj + b_proj  # [C]
    out = np.empty((B, T, C), dtype=np.float32)
    for b in range(B):
        out[b] = (res.results[2 * b]["y"].astype(np.float32)
                  + res.results[2 * b + 1]["y"].astype(np.float32) + correction)
    return out


# revision 26
# speedup vs baseline: 1.3647x; 1.0209x over previous
"""Causal self-attention on 8 trn2 NeuronCores.

Full inputs in, full output out. Sharding: data-parallel over batch (B=4),
tensor-parallel over head groups (16 heads -> 2 groups of 8). core = 2*b + g.

Per-core math (T=2048, C=1024, 8 heads, D=64, group channels G=512):
  qT/kT: [64*(h%2)+d, h//2, t] layout so scores need no transposes
  scoresT[j,i] = sum_d kT[d,j] qT[d,i]   (q pre-scaled by 1/sqrt(D) on host)
  softmax without max-subtraction (scores ~ N(0,1) by construction; exp is
  exactly shift-invariant so this matches the reference softmax)
  expT row sums come free from an all-ones 65th column appended to V
  causal mask applied post-exp by multiplying the diagonal 128x128 block
  with a precomputed lower-triangle 0/1 tile (DVE, keeps gpsimd off the
  critical exp->out chain)
  out_T[d,i] = sum_j v[j,d] expT[j,i]; normalize by 1/sums; y = oT.T @ Wp

Fully fused single-region pipeline: per 512-token chunk tch we emit
QKV projections for chunk tch, then attention rows ic=tch for all head
pairs (keys 0..tch only -- causal), then the output projection for those
tokens. The Tile list scheduler then overlaps phase boundaries (exp on
Scalar, copies on DVE, r-broadcasts on GpSimd, projections filling PE
gaps). Normalization is per (pair, ic) so the kernel tail is short.

PSUM budget (8 banks): proj pool 2x[128,512] (qkv + y outputs), scores
2x[128,2,512], out-accum 2x[65,512].

Host gather: y[b] = part[2b] + part[2b+1] + b_attn_v @ W_proj + b_proj
(q/k biases are added on-device; the v bias commutes through softmax).
"""

import numpy as np
from contextlib import ExitStack

import concourse.bass as bass
import concourse.tile as tile
from concourse import bacc, mybir
from concourse.bass_utils import run_bass_kernel_spmd

P = 128
B, T, C, H = 4, 2048, 1024, 16
D = 64
HG = 8          # heads per core
G = HG * D      # 512 head channels per core
CT = C // P     # 8 contraction tiles
TCH = T // 512  # 4 chunks of 512 tokens
NT = G // P     # 4 tiles of head channels

f32 = mybir.dt.float32
bf16 = mybir.dt.bfloat16
MM_DT = bf16
EXP_DT = bf16


def build_attention(nc: bass.Bass):
    xT = nc.dram_tensor("xT", [C, T], MM_DT, kind="ExternalInput")
    wq = nc.dram_tensor("wq", [C, G], MM_DT, kind="ExternalInput")
    wk = nc.dram_tensor("wk", [C, G], MM_DT, kind="ExternalInput")
    wv = nc.dram_tensor("wv", [C, G], MM_DT, kind="ExternalInput")
    wp = nc.dram_tensor("wp", [G, C], MM_DT, kind="ExternalInput")
    bq = nc.dram_tensor("bq", [P, NT], f32, kind="ExternalInput")
    bk = nc.dram_tensor("bk", [P, NT], f32, kind="ExternalInput")
    y = nc.dram_tensor("y", [T, C], f32, kind="ExternalOutput")

    with tile.TileContext(nc) as tc, ExitStack() as ctx:
        persist = ctx.enter_context(tc.tile_pool(name="persist", bufs=1))
        qT = persist.tile([P, NT, T], MM_DT)
        kT = persist.tile([P, NT, T], MM_DT)
        v_aug = persist.tile([P, T // P, HG, D + 1], MM_DT)
        oT = persist.tile([P, NT, T], MM_DT)
        wq_sb = persist.tile([P, CT, G], MM_DT)
        wk_sb = persist.tile([P, CT, G], MM_DT)
        wv_sb = persist.tile([P, CT, G], MM_DT)
        wp_sb = persist.tile([P, NT, C], MM_DT)
        bq_sb = persist.tile([P, NT], f32)
        bk_sb = persist.tile([P, NT], f32)
        tri = persist.tile([P, 1, P], EXP_DT)

        # weight DMAs, split per-ct so they spread across DMA engines
        # (one dma_start lands on ONE engine at ~22 GB/s; 1MB would be 44us).
        # wq first: first matmuls need it; wp needed last.
        for ct in range(CT):
            nc.sync.dma_start(out=wq_sb[:, ct, :], in_=wq.ap()[P * ct:P * (ct + 1), :])
        xpool = ctx.enter_context(tc.tile_pool(name="xpool", bufs=2))
        xc0 = xpool.tile([P, CT, 512], MM_DT, tag="xc", name="xc_0")
        for ct in range(CT):
            nc.sync.dma_start(out=xc0[:, ct, :],
                              in_=xT.ap()[P * ct:P * (ct + 1), 0:512])
        for ct in range(CT):
            nc.sync.dma_start(out=wk_sb[:, ct, :], in_=wk.ap()[P * ct:P * (ct + 1), :])
        for ct in range(CT):
            nc.sync.dma_start(out=wv_sb[:, ct, :], in_=wv.ap()[P * ct:P * (ct + 1), :])
        nc.sync.dma_start(out=bq_sb, in_=bq.ap())
        nc.sync.dma_start(out=bk_sb, in_=bk.ap())
        for nt in range(NT):
            nc.sync.dma_start(out=wp_sb[:, nt, :], in_=wp.ap()[P * nt:P * (nt + 1), :])

        # constants: ones column of v_aug; lower-triangle mask (tri[p,x]=x>=p)
        ones_col = persist.tile([P, 1], f32)
        nc.vector.memset(ones_col, 1.0)
        nc.vector.tensor_copy(
            out=v_aug[:, :, :, D:D + 1],
            in_=ones_col.to_broadcast([P, T // P, HG, 1]),
        )
        nc.vector.memset(tri, 1.0)
        nc.gpsimd.affine_select(
            out=tri[:, 0, :], in_=tri[:, 0, :], compare_op=mybir.AluOpType.is_ge,
            fill=0.0, base=0, channel_multiplier=-1, pattern=[[1, P]],
        )

        epool = ctx.enter_context(tc.tile_pool(name="epool", bufs=6))
        upool = ctx.enter_context(tc.tile_pool(name="upool", bufs=10))
        rpool = ctx.enter_context(tc.tile_pool(name="rpool", bufs=2))
        bpool = ctx.enter_context(tc.tile_pool(name="bpool", bufs=4))
        ypool = ctx.enter_context(tc.tile_pool(name="ypool", bufs=3))
        ps_proj = ctx.enter_context(tc.tile_pool(name="ps_proj", bufs=2, space="PSUM"))
        ps_s = ctx.enter_context(tc.tile_pool(name="ps_s", bufs=2, space="PSUM"))
        ps_o = ctx.enter_context(tc.tile_pool(name="ps_o", bufs=2, space="PSUM"))

        xT_r = xT.ap().rearrange("(ct p) t -> p ct t", p=P)

        def emit_phase3(ic):
            # output projection for chunk ic (emitted one iteration late so
            # its psum-slot claims don't block the next chunk's projections
            # while waiting on oT)
            for tt4 in range(4):
                tt = 4 * ic + tt4
                y_sb = ypool.tile([P, C], f32, tag="ysb", name=f"ysb_{tt}")
                for mc in range(C // 512):
                    y_ps = ps_proj.tile([P, 512], f32, tag="proj",
                                        name=f"y_{tt}_{mc}")
                    for nt in range(NT):
                        nc.tensor.matmul(
                            y_ps,
                            oT[:, nt, P * tt:P * (tt + 1)],
                            wp_sb[:, nt, 512 * mc:512 * (mc + 1)],
                            start=(nt == 0), stop=(nt == NT - 1),
                        )
                    nc.vector.tensor_copy(out=y_sb[:, 512 * mc:512 * (mc + 1)],
                                          in_=y_ps)
                nc.sync.dma_start(out=y.ap()[P * tt:P * (tt + 1), :], in_=y_sb)

        for tch in range(TCH):
            # ---------------- QKV projections for chunk tch ----------------
            if tch == 0:
                xc = xc0
            else:
                xc = xpool.tile([P, CT, 512], MM_DT, tag="xc", name=f"xc_{tch}")
                for ct in range(CT):
                    nc.sync.dma_start(
                        out=xc[:, ct, :],
                        in_=xT.ap()[P * ct:P * (ct + 1), 512 * tch:512 * (tch + 1)])

            for w_sb, b_sb, dstT in ((wq_sb, bq_sb, qT), (wk_sb, bk_sb, kT)):
                for jt in range(NT):
                    ps = ps_proj.tile([P, 512], f32, tag="proj",
                                      name=f"pqk_{tch}_{jt}")
                    for ct in range(CT):
                        nc.tensor.matmul(
                            ps, w_sb[:, ct, P * jt:P * (jt + 1)], xc[:, ct, :],
                            start=(ct == 0), stop=(ct == CT - 1),
                        )
                    nc.vector.tensor_scalar_add(
                        out=dstT[:, jt, 512 * tch:512 * (tch + 1)],
                        in0=ps, scalar1=b_sb[:, jt:jt + 1],
                    )
            for tt4 in range(4):
                tt = 4 * tch + tt4
                ps = ps_proj.tile([P, 512], f32, tag="proj", name=f"pv_{tt}")
                for ct in range(CT):
                    nc.tensor.matmul(
                        ps, xc[:, ct, P * tt4:P * (tt4 + 1)], wv_sb[:, ct, :],
                        start=(ct == 0), stop=(ct == CT - 1),
                    )
                nc.vector.tensor_copy(
                    out=v_aug[:, tt, :, 0:D],
                    in_=ps.rearrange("p (h d) -> p h d", h=HG),
                )

            if tch > 0:
                emit_phase3(tch - 1)

            # ---------------- attention rows ic = tch ----------------
            # Sum rows are staged at 32-aligned partitions {0,32,64,96} so
            # gpsimd partition_broadcast can read them (Q7 quadrant rule),
            # while one DVE reciprocal instruction covers 4 rows at once
            # (recip is an 8-pass iterative op costing free-size regardless
            # of partition count -- batching by partitions is free).
            ic = tch
            n_jb = 4 * ic + 4
            last_ic = (ic == TCH - 1)
            if not last_ic:
                S_ic = rpool.tile([8, 512], f32, tag="S", name=f"S_{ic}")
                R_ic = rpool.tile([8, 512], f32, tag="R", name=f"R_{ic}")
            o_us = {}
            for g2 in range(HG // 2):  # head pairs, nt = g2
                o_ps = {}
                for hh in range(2):
                    o_ps[hh] = ps_o.tile([D + 1, 512], f32, tag="o",
                                         name=f"ops_{2 * g2 + hh}_{ic}")
                for jb in range(n_jb):
                    off = max(0, P * jb - 512 * ic)
                    s = ps_s.tile([P, 2, 512], f32, tag="s",
                                  name=f"sps_{g2}_{ic}_{jb}")
                    for hh in range(2):
                        band = 64 * hh
                        nc.tensor.matmul(
                            s[:, hh, off:],
                            kT[band:band + D, g2, P * jb:P * (jb + 1)],
                            qT[band:band + D, g2, 512 * ic + off:512 * (ic + 1)],
                            start=True, stop=True,
                        )
                    e = epool.tile([P, 2, 512], EXP_DT, tag="e",
                                   name=f"e_{g2}_{ic}_{jb}")
                    nc.scalar.activation(
                        out=e[:, :, off:], in_=s[:, :, off:],
                        func=mybir.ActivationFunctionType.Exp,
                    )
                    if P * jb >= 512 * ic:  # diagonal triangle mask
                        nc.vector.tensor_mul(
                            out=e[:, :, off:off + P],
                            in0=e[:, :, off:off + P],
                            in1=tri.to_broadcast([P, 2, P]),
                        )
                    for hh in range(2):
                        h = 2 * g2 + hh
                        nc.tensor.matmul(
                            o_ps[hh][:, off:], v_aug[:, jb, h, :],
                            e[:, hh, off:],
                            start=(jb == 0), stop=(jb == n_jb - 1),
                        )
                # drain psum; gather sum rows (last chunk: per-pair recip so
                # the kernel tail chain is short; earlier: batched per ic)
                for hh in range(2):
                    h = 2 * g2 + hh
                    o_u = upool.tile([D + 1, 512], f32, tag="ou",
                                     name=f"ou_{h}_{ic}")
                    nc.vector.tensor_copy(o_u, o_ps[hh])
                    o_us[h] = o_u
                if last_ic:
                    # per-pair normalize: short kernel tail (baseline-proven
                    # mechanics: DMA row gather, dense recip, base-0 bcast)
                    S_g = rpool.tile([2, 512], f32, tag="Sg", name=f"Sg_{g2}")
                    R_g = rpool.tile([2, 512], f32, tag="Rg", name=f"Rg_{g2}")
                    for hh in range(2):
                        nc.sync.dma_start(out=S_g[hh:hh + 1, :],
                                          in_=o_us[2 * g2 + hh][D:D + 1, :])
                    nc.vector.reciprocal(R_g, S_g)
                    for hh in range(2):
                        h = 2 * g2 + hh
                        rrow = bpool.tile([1, 512], f32, tag="rrow",
                                          name=f"rr_{h}_{ic}")
                        nc.sync.dma_start(out=rrow, in_=R_g[hh:hh + 1, :])
                        rb = bpool.tile([D, 512], f32, tag="rb",
                                        name=f"rb_{h}_{ic}")
                        nc.gpsimd.partition_broadcast(rb, rrow[0:1, :])
                        nc.vector.tensor_mul(
                            out=oT[64 * (h % 2):64 * (h % 2) + D, h // 2,
                                   512 * ic:512 * (ic + 1)],
                            in0=o_us[h][0:D, :],
                            in1=rb,
                        )
                else:
                    for hh in range(2):
                        h = 2 * g2 + hh
                        nc.sync.dma_start(out=S_ic[h:h + 1, :],
                                          in_=o_us[h][D:D + 1, :])
            if not last_ic:
                nc.vector.reciprocal(R_ic, S_ic)
                for h in range(HG):
                    rrow = bpool.tile([1, 512], f32, tag="rrow",
                                      name=f"rr_{h}_{ic}")
                    nc.sync.dma_start(out=rrow, in_=R_ic[h:h + 1, :])
                    rb = bpool.tile([D, 512], f32, tag="rb", name=f"rb_{h}_{ic}")
                    nc.gpsimd.partition_broadcast(rb, rrow[0:1, :])
                    nc.vector.tensor_mul(
                        out=oT[64 * (h % 2):64 * (h % 2) + D, h // 2,
                               512 * ic:512 * (ic + 1)],
                        in0=o_us[h][0:D, :],
                        in1=rb,
                    )

        emit_phase3(TCH - 1)


_NC_CACHE = {}


def _get_nc():
    if "nc" not in _NC_CACHE:
        nc = bacc.Bacc("TRN2", debug=False, num_devices=8)
        build_attention(nc)
        nc.compile()
        _NC_CACHE["nc"] = nc
    return _NC_CACHE["nc"]


def kernel(x, W_attn, b_attn, W_proj, b_proj):
    x = np.asarray(x, dtype=np.float32)
    W_attn = np.asarray(W_attn, dtype=np.float32)
    b_attn = np.asarray(b_attn, dtype=np.float32)
    W_proj = np.asarray(W_proj, dtype=np.float32)
    b_proj = np.asarray(b_proj, dtype=np.float32)

    import ml_dtypes
    mm_np = ml_dtypes.bfloat16

    scale = 1.0 / np.sqrt(np.float32(D))
    in_maps = []
    for core in range(8):
        b, g = divmod(core, 2)
        cols = slice(G * g, G * (g + 1))
        bqs = (b_attn[0:C][cols] * scale).reshape(NT, 2, D).transpose(1, 2, 0).reshape(P, NT)
        bks = b_attn[C:2 * C][cols].reshape(NT, 2, D).transpose(1, 2, 0).reshape(P, NT)
        in_maps.append({
            "xT": np.ascontiguousarray(x[b].T).astype(mm_np),
            "wq": np.ascontiguousarray(W_attn[:, 0:C][:, cols] * scale).astype(mm_np),
            "wk": np.ascontiguousarray(W_attn[:, C:2 * C][:, cols]).astype(mm_np),
            "wv": np.ascontiguousarray(W_attn[:, 2 * C:3 * C][:, cols]).astype(mm_np),
            "wp": np.ascontiguousarray(W_proj[G * g:G * (g + 1), :]).astype(mm_np),
            "bq": np.ascontiguousarray(bqs),
            "bk": np.ascontiguousarray(bks),
        })

    res = run_bass_kernel_spmd(_get_nc(), in_maps, core_ids=list(range(8)))

    correction = b_attn[2 * C:3 * C] @ W_proj + b_proj  # [C]
    out = np.empty((B, T, C), dtype=np.float32)
    for b in range(B):
        out[b] = (res.results[2 * b]["y"].astype(np.float32)
                  + res.results[2 * b + 1]["y"].astype(np.float32) + correction)
    return out
